# revision 1
# baseline (speedup 1.0000x reference)
"""Embedding-lookup + row-wise dot kernel for Trainium2 (8 NeuronCores).

Problem (hardcoded, self-contained):
    users:       [16384] int   (values < 1_000_000)
    movies:      [16384] int   (values < 100_000)
    user_table:  [1_000_000, 64] f32
    movie_table: [100_000, 64] f32
    out = sum(user_table[users] * movie_table[movies], axis=-1, keepdims=True)
        -> [16384, 1] f32

Sharding: data-parallel — tables replicated on all 8 cores, batch split
into 8 x 2048. Each core gathers 2048 user rows + 2048 movie rows via
vector-indirect DMAs (128 rows / 256 B each per instruction — the only
HW-supported shape: one int32 index per output partition), with the DVE
multiply+reduce and SP output stores pipelined behind the serialized
Pool-engine gather stream. The gather stream (32 instructions x ~1.41 us)
is the hard floor on this runtime: the faster InstDMAGatherAnt path needs
the custom Q7 ucode library that bedrock images exclude, and multi-index
offset APs hard-crash the device.
"""

import os
import numpy as np

N_USERS = 1_000_000
N_MOVIES = 100_000
EMB = 64
BATCH = 16384
N_CORES = 8
P = 128
B_CORE = BATCH // N_CORES  # 2048
J = B_CORE // P            # 16

_NC_CACHE = {}


def _build_nc():
    """Build the per-core Bass program (same program on all 8 cores)."""
    import concourse.bacc as bacc
    import concourse.bass as bass
    import concourse.tile as tile
    from concourse import mybir

    nc = bacc.Bacc(None, target_bir_lowering=False)

    users_t = nc.dram_tensor("users", [P, J], mybir.dt.int32, kind="ExternalInput")
    movies_t = nc.dram_tensor("movies", [P, J], mybir.dt.int32, kind="ExternalInput")
    user_table_t = nc.dram_tensor(
        "user_table", [N_USERS, EMB], mybir.dt.float32, kind="ExternalInput"
    )
    movie_table_t = nc.dram_tensor(
        "movie_table", [N_MOVIES, EMB], mybir.dt.float32, kind="ExternalInput"
    )
    out_t = nc.dram_tensor("out", [P, J], mybir.dt.float32, kind="ExternalOutput")

    GROUPS = [2, 4, 4, 5, 1]  # small first group: shrinks the critical first
    # index load; small last group: minimizes exposed tail compute
    assert sum(GROUPS) == J
    G0 = GROUPS[0]

    with tile.TileContext(nc) as tc:
        with tc.tile_pool(name="sbuf", bufs=1) as sbuf:
            # First group's user indices load first (tiny DMA) so the first
            # gathers only wait on it; movie + remaining user indices load
            # behind it and their completion hides under the gather stream.
            u_idx_h = sbuf.tile([P, G0], mybir.dt.int32)
            m_idx = sbuf.tile([P, J], mybir.dt.int32)
            u_idx_t = sbuf.tile([P, J - G0], mybir.dt.int32)
            nc.sync.dma_start(out=u_idx_h[:], in_=users_t[:, :G0])
            nc.sync.dma_start(out=m_idx[:], in_=movies_t[:])
            nc.sync.dma_start(out=u_idx_t[:], in_=users_t[:, G0:])

            def u_idx_col(j):
                if j < G0:
                    return u_idx_h[:, j : j + 1]
                return u_idx_t[:, j - G0 : j - G0 + 1]

            # Gather 128 rows per indirect DMA (one index per partition — the
            # only HW-supported shape). Per-group tiles so DVE mul/reduce and
            # the SP output store pipeline behind the Pool gather stream.
            # Within a group: all u-gathers, then all m-gathers, so the
            # m-index load latency hides behind the u-gathers.
            col = 0
            for g, GC in enumerate(GROUPS):
                u_g = sbuf.tile([P, GC, EMB], mybir.dt.float32, tag=f"u{g}")
                m_g = sbuf.tile([P, GC, EMB], mybir.dt.float32, tag=f"m{g}")
                for jj in range(GC):
                    nc.gpsimd.indirect_dma_start(
                        out=u_g[:, jj, :],
                        out_offset=None,
                        in_=user_table_t[:],
                        in_offset=bass.IndirectOffsetOnAxis(
                            ap=u_idx_col(col + jj), axis=0
                        ),
                        oob_is_err=False,
                    )
                for jj in range(GC):
                    j = col + jj
                    nc.gpsimd.indirect_dma_start(
                        out=m_g[:, jj, :],
                        out_offset=None,
                        in_=movie_table_t[:],
                        in_offset=bass.IndirectOffsetOnAxis(
                            ap=m_idx[:, j : j + 1], axis=0
                        ),
                        oob_is_err=False,
                    )
                prod_g = sbuf.tile([P, GC, EMB], mybir.dt.float32, tag=f"p{g}")
                nc.vector.tensor_mul(out=prod_g[:], in0=u_g[:], in1=m_g[:])
                res_g = sbuf.tile([P, GC], mybir.dt.float32, tag=f"r{g}")
                nc.vector.tensor_reduce(
                    out=res_g[:],
                    in_=prod_g[:],
                    axis=mybir.AxisListType.X,
                    op=mybir.AluOpType.add,
                )
                nc.sync.dma_start(out=out_t[:, col : col + GC], in_=res_g[:])
                col += GC

    nc.compile()
    return nc


def _install_ntff_hook():
    """Shim antenv.axon_hooks (absent in this image) so trace=True works
    under axon, and disable the S3 artifact upload (zero-egress container)."""
    import sys
    import types

    import concourse.bass_utils as bu

    bu.upload_artifacts = lambda d: d

    try:
        from antenv.axon_hooks import get_axon_ntff_profile_hook  # noqa: F401

        return
    except ImportError:
        pass

    import antenv
    from trn_agent_boot.trn_boot import _ntff_profile_via_ctypes

    mod = types.ModuleType("antenv.axon_hooks")
    mod._hook = _ntff_profile_via_ctypes("/opt/axon/libaxon_pjrt.so")
    mod.set_axon_ntff_profile_hook = lambda h: setattr(mod, "_hook", h)
    mod.get_axon_ntff_profile_hook = lambda: mod._hook
    sys.modules["antenv.axon_hooks"] = mod
    antenv.axon_hooks = mod


def kernel(users, movies, user_table, movie_table):
    from concourse.bass_utils import run_bass_kernel_spmd

    users = np.ascontiguousarray(np.asarray(users).astype(np.int32))
    movies = np.ascontiguousarray(np.asarray(movies).astype(np.int32))
    user_table = np.ascontiguousarray(np.asarray(user_table, dtype=np.float32))
    movie_table = np.ascontiguousarray(np.asarray(movie_table, dtype=np.float32))

    if "nc" not in _NC_CACHE:
        _NC_CACHE["nc"] = _build_nc()
    nc = _NC_CACHE["nc"]

    in_maps = []
    for c in range(N_CORES):
        sl = slice(c * B_CORE, (c + 1) * B_CORE)
        # idx tile column j = batch group j (128 consecutive elements):
        # host array [P, J] with arr[p, j] = batch[j*128 + p]
        in_maps.append(
            {
                "users": np.ascontiguousarray(users[sl].reshape(J, P).T),
                "movies": np.ascontiguousarray(movies[sl].reshape(J, P).T),
                "user_table": user_table,
                "movie_table": movie_table,
            }
        )

    trace = bool(os.environ.get("KERNEL_TRACE"))
    if trace:
        try:
            _install_ntff_hook()
        except Exception:
            trace = False
    res = run_bass_kernel_spmd(
        nc, in_maps, core_ids=list(range(N_CORES)), trace=trace
    )
    if trace:
        kernel.last_exec_time_ns = res.exec_time_ns
        kernel.last_trace = res.instructions_and_trace

    # res tile [P, J]: column j = batch group j -> transpose back
    out = np.concatenate(
        [np.ascontiguousarray(r["out"].T).reshape(B_CORE) for r in res.results]
    )
    return out.reshape(BATCH, 1).astype(np.float32)

